# revision 1
# baseline (speedup 1.0000x reference)
"""KNRM-KG ranker kernel for 8 Trainium2 NeuronCores (Bass/Tile).

Strategy: pure data parallel over the batch dim (64 = 8 cores x 8 batches).
Device-side pipeline per batch:
  - transposed dma_gather of word embeddings from a per-core vocabulary-
    compacted fp16 table (rows padded to 384 cols) -> reps arrive with the
    word dim on partitions (no on-device transposes needed)
  - fp16 matmuls accumulate word + entity transforms in PSUM; bias+relu are
    fused into the PSUM->SBUF copy (tensor_scalar add+max on DVE for ah=0,
    Relu activation on Act for ah=1 - balances the two engines)
  - doc norms: d2 = dts^2 on DVE, then att-sum via PE (per 128-doc chunk an
    accumulating matmul pair lhsT=d2-chunk, rhs=ones column) -> nd lands in
    PSUM with doc on partitions; rd = exp(-0.5*ln(nd)) * dmask on Act/DVE
  - simT = d^T-chunk (stationary) @ qhat (moving) per doc chunk; the
    PSUM->SBUF copy applies rd per partition and reorders (c,q)->(q,c)
  - RBF bank: shared E0 = exp(-50 s^2), per-mu Fk = exp(100 mu s - 50 mu^2),
    Gk = E0*Fk on VectorE; sigma=0.001 kernel via Square+Exp; the doc-sum of
    each kernel runs on PE: 16 accumulating matmuls (lhsT = Gk chunk strided,
    rhs = ones column) -> PSUM column per kernel, (r,q) on partitions
  - tail per block: ln/W_c-dot/mask on [rq, 11] tiles, q-sum via one tiny
    matmul into the scores row
  - RBF work of block i is interleaved between the batches of block i+1
    (software pipelining; each engine executes its queue in order, so without
    interleaving PE idles during RBF and DVE/Act idle during batch phases)
"""

import numpy as np

# ---------------------------------------------------------------- constants
B, QLEN, DLEN = 64, 32, 2048
VOCAB, WORD, ENT, ATT = 50000, 300, 128, 256
NCORES = 8
BPC = B // NCORES            # batches per core
WPAD = 384                   # padded word dim (3 x 128, 768B fp16 rows)
NWCH = 3                     # word chunks of 128
NAH = 2                      # att halves of 128
NDC = DLEN // 512            # doc 512-chunks (4)
NC16 = DLEN // 128           # doc 128-chunks (16)
MUS = [-0.9, -0.7, -0.5, -0.3, -0.1, 0.1, 0.3, 0.5, 0.7, 0.9]  # sigma=0.1
RBF_PACK = 4                 # batches per RBF block
GSZ = 512                    # idxs per dma_gather (HW cap)

# packed fp16 constant block offsets
C16_WT, C16_WE, C16_QE, C16_ON, C16_WC = 0, 768, 1024, 1280, 1408
C16_TOT = 1424
# packed f32 constant block offsets
C32_BIAS, C32_DM, C32_BM, C32_ACT = 0, 2, 130, 134
C32_TOT = 147

_CACHE = {}


# ---------------------------------------------------------------- program
BLOCKS = [2, 2, 2, 2]


def _build_program(spk=True, pack=RBF_PACK, blocks=None, nreps=1,
                   ablate=(), gsz=GSZ, scratch=16384):
    import concourse.bacc as bacc
    import concourse.mybir as mybir
    import concourse.tile as tile

    fp16 = mybir.dt.float16
    f32 = mybir.dt.float32
    i16 = mybir.dt.int16
    bf16 = mybir.dt.bfloat16
    AF = mybir.ActivationFunctionType
    ALU = mybir.AluOpType
    AX = mybir.AxisListType

    if blocks is None:
        blocks = list(BLOCKS)
    assert sum(blocks) == BPC

    # Pin all activations to the one table set covering Exp/Ln/Square so the
    # act-table pass emits a single load instead of thrashing between sets.
    import concourse.hw_specs as hw_specs
    _orig_tables = hw_specs.get_activation_tables

    def _one_set(arch):
        t = _orig_tables(arch)
        return {k: (v if k == "natural_log_exp_and_others" else frozenset())
                for k, v in t.items()}

    hw_specs.get_activation_tables = _one_set
    import concourse.bacc as _bacc_mod
    _bacc_mod.get_activation_tables = _one_set
    nc = bacc.Bacc("TRN2", target_bir_lowering=False, debug=False,
                   num_devices=NCORES, dynamic_dma_scratch_size=scratch)

    # DRAM inputs (per-core)
    d_tab = nc.dram_tensor("tab", [17000, WPAD], fp16, kind="ExternalInput")
    d_ci16 = nc.dram_tensor("ci16", [128, BPC * 128 + 16], i16,
                            kind="ExternalInput")
    d_cf16 = nc.dram_tensor("cf16", [128, C16_TOT], fp16,
                            kind="ExternalInput")
    d_cf32 = nc.dram_tensor("cf32", [128, C32_TOT], f32,
                            kind="ExternalInput")
    d_entT = nc.dram_tensor("entT", [BPC, 128, DLEN], fp16,
                            kind="ExternalInput")
    d_qrow = nc.dram_tensor("qrow", [1, 257], f32, kind="ExternalInput")
    d_onesf = nc.dram_tensor("onesf", [1, 128], fp16, kind="ExternalInput")
    d_out = nc.dram_tensor("out", [1, BPC], f32, kind="ExternalOutput")

    with tile.TileContext(nc) as tc:
        with (
            tc.tile_pool(name="const", bufs=1) as cpool,
            tc.tile_pool(name="gath", bufs=2) as gpool,
            tc.tile_pool(name="ent", bufs=2) as epool,
            tc.tile_pool(name="dt", bufs=2) as dpool,
            tc.tile_pool(name="work", bufs=2) as wpool,
            tc.tile_pool(name="rbf", bufs=2) as rpool,
            tc.tile_pool(name="fin", bufs=2) as fpool,
            tc.tile_pool(name="pt", bufs=2, space="PSUM") as pt,   # transform
            tc.tile_pool(name="ps", bufs=1, space="PSUM") as ps,   # simT
            tc.tile_pool(name="pn", bufs=1, space="PSUM") as pn,   # norms
            tc.tile_pool(name="pr", bufs=1, space="PSUM") as pr,   # misc
            tc.tile_pool(name="pk", bufs=1, space="PSUM") as pk,   # rbf sums
        ):
            # ---------------- load constants (packed by dtype)
            ci16 = cpool.tile([128, BPC * 128 + 16], i16)
            nc.sync.dma_start(out=ci16[:], in_=d_ci16.ap())
            cf16 = cpool.tile([128, C16_TOT], fp16)
            nc.sync.dma_start(out=cf16[:], in_=d_cf16.ap())
            cf32 = cpool.tile([128, C32_TOT], f32)
            nc.sync.dma_start(out=cf32[:], in_=d_cf32.ap())
            qrow = cpool.tile([1, 257], f32)
            nc.sync.dma_start(out=qrow[:], in_=d_qrow.ap())
            onesf = cpool.tile([1, 128], fp16)
            nc.sync.dma_start(out=onesf[:], in_=d_onesf.ap())

            idxd = ci16[:, 0:BPC * 128]
            idxq = ci16[:, BPC * 128:BPC * 128 + 16]
            we = cf16[:, C16_WE:C16_WE + ATT]
            qentT = cf16[:, C16_QE:C16_QE + BPC * QLEN]
            ones16 = cf16[:, C16_ON:C16_ON + 128]
            wcb = cf16[:, C16_WC:C16_WC + 11]
            bias = cf32[:, C32_BIAS:C32_BIAS + NAH]
            dmask = cf32[:, C32_DM:C32_DM + BPC * NC16]
            bmask = cf32[:, C32_BM:C32_BM + 4]
            actb = cf32[:, C32_ACT:C32_ACT + 13]
            qmask = qrow[:, 0:BPC * QLEN]
            bc = qrow[:, 256:257]

            def wt_l(wc, ah):  # W_t lhsT block [128 word, 128 att]
                o = C16_WT + (wc * NAH + ah) * 128
                return cf16[:, o:o + 128]
            for _rep in range(nreps):

                # ---------------- query side (once per core)
                qg = cpool.tile([128, NWCH, BPC * QLEN], fp16)
                if "qg" in ablate:
                    nc.gpsimd.memset(qg[:], 0.01)
                else:
                    nc.gpsimd.dma_gather(
                        out_ap=qg[:], in_ap=d_tab.ap(), idxs_ap=idxq,
                        num_idxs=BPC * QLEN, num_idxs_reg=BPC * QLEN,
                        elem_size=WPAD, transpose=True, single_packet=spk)

                qhat = cpool.tile([128, NAH, BPC * QLEN], fp16)
                q2 = wpool.tile([128, BPC * QLEN], fp16, tag="q2")
                nq_ps = pr.tile([1, BPC * QLEN], f32, tag="small")
                for ah in range(NAH):
                    qp = pt.tile([128, 512], f32, tag="tps")
                    for wc in range(NWCH):
                        nc.tensor.matmul(qp[:, :BPC * QLEN], lhsT=wt_l(wc, ah),
                                         rhs=qg[:, wc, :], start=(wc == 0),
                                         stop=False)
                    nc.tensor.matmul(qp[:, :BPC * QLEN],
                                     lhsT=we[:, ah * 128:(ah + 1) * 128],
                                     rhs=qentT, start=False, stop=True)
                    # bias + relu fused
                    nc.vector.tensor_scalar(
                        out=qhat[:, ah, :], in0=qp[:, :BPC * QLEN],
                        scalar1=bias[:, ah:ah + 1], scalar2=0.0,
                        op0=ALU.add, op1=ALU.max)
                    nc.vector.tensor_tensor(out=q2[:], in0=qhat[:, ah, :],
                                            in1=qhat[:, ah, :], op=ALU.mult)
                    nc.tensor.matmul(nq_ps[:], lhsT=ones16[:, 0:1], rhs=q2[:],
                                     start=(ah == 0), stop=(ah == 1))
                # rq = exp(-0.5*ln(nq)) * qmask  (single partition, 256 elems)
                rq = cpool.tile([1, BPC * QLEN], f32)
                nc.scalar.activation(rq[:], nq_ps[:], AF.Ln, bias=actb[0:1, 0:1])
                nc.scalar.activation(rq[:], rq[:], AF.Exp, scale=-0.5)
                nc.vector.tensor_tensor(out=rq[:], in0=rq[:], in1=qmask,
                                        op=ALU.mult)
                rq16 = cpool.tile([1, BPC * QLEN], fp16)
                nc.vector.tensor_copy(rq16[:], rq[:])
                rqb_ps = pr.tile([128, BPC * QLEN], f32, tag="small")
                nc.tensor.matmul(rqb_ps[:], lhsT=onesf[:], rhs=rq16[:],
                                 start=True, stop=True)
                for ah in range(NAH):
                    nc.vector.tensor_tensor(out=qhat[:, ah, :], in0=qhat[:, ah, :],
                                            in1=rqb_ps[:], op=ALU.mult)

                # ---------------- main loop: batches + pipelined RBF
                scores = cpool.tile([1, BPC], f32)
                if "rbf" in ablate or "docsum" in ablate:
                    nc.gpsimd.memset(scores[:], 0.0)
                pending = []       # RBF closures of the previous block

                def drain(k):
                    for _ in range(min(k, len(pending))):
                        pending.pop(0)()

                ents = {}

                def fetch_ent(b):
                    # prefetch entity reps one batch ahead so the DMA
                    # latency hides under the previous batch's transform
                    if b < BPC and b not in ents:
                        t = epool.tile([128, DLEN], fp16, tag="ent",
                                       name="ent")
                        nc.sync.dma_start(out=t[:], in_=d_entT.ap()[b])
                        ents[b] = t

                def batch_work(b, rb, S, Sv):
                    # gather doc reps (transposed), GSZ idxs per instruction
                    gs = []
                    for gh in range(DLEN // gsz):
                        g = gpool.tile([128, NWCH, gsz], fp16,
                                       tag=f"g{gh}", name=f"g{gh}")
                        o = b * 128 + gh * (gsz // 16)
                        nc.gpsimd.dma_gather(
                            out_ap=g[:], in_ap=d_tab.ap(),
                            idxs_ap=idxd[:, o:o + gsz // 16],
                            num_idxs=gsz, num_idxs_reg=gsz,
                            elem_size=WPAD, transpose=True, single_packet=spk)
                        gs.append(g)
                    fetch_ent(b)
                    ent = ents.pop(b)
                    fetch_ent(b + 1)

                    dts = [dpool.tile([128, DLEN], fp16, tag=f"dt{ah}",
                                      name=f"dt{ah}") for ah in range(NAH)]
                    nd_ps = pn.tile([128, NC16], f32, tag="ndps", name="nd_ps")
                    d2s = []
                    for pp in range(NDC // 2):
                        dcs = (2 * pp, 2 * pp + 1)
                        sls = [slice(dc * 512, (dc + 1) * 512) for dc in dcs]
                        pr_sl = slice(sls[0].start, sls[1].stop)
                        for ah in range(NAH):
                            # one 2-bank PSUM tile per pp; matmuls target the
                            # in-bank halves, bias+relu reads span both banks
                            tps = pt.tile([128, 1024], f32, tag="tps",
                                          name="tps")
                            for wc in range(NWCH):
                                for j, dc in enumerate(dcs):
                                    gv = gs[dc * 512 // gsz]
                                    go = dc * 512 % gsz
                                    nc.tensor.matmul(
                                        tps[:, j * 512:(j + 1) * 512],
                                        lhsT=wt_l(wc, ah),
                                        rhs=gv[:, wc, go:go + 512],
                                        start=(wc == 0), stop=False)
                            for j, dc in enumerate(dcs):
                                nc.tensor.matmul(
                                    tps[:, j * 512:(j + 1) * 512],
                                    lhsT=we[:, ah * 128:(ah + 1) * 128],
                                    rhs=ent[:, sls[j]], start=False, stop=True)
                            if ah == 0:
                                nc.vector.tensor_scalar(
                                    out=dts[ah][:, pr_sl], in0=tps[:],
                                    scalar1=bias[:, ah:ah + 1],
                                    scalar2=0.0, op0=ALU.add, op1=ALU.max)
                            else:
                                nc.scalar.activation(
                                    dts[ah][:, pr_sl], tps[:],
                                    AF.Relu, bias=bias[:, ah:ah + 1])
                            d2 = wpool.tile([128, 1024], fp16, tag="d2",
                                            name="d2")
                            d2s.append(d2)
                            d2_eng = (nc.gpsimd if (ah == 1 and pp == 0)
                                      else nc.vector)
                            d2_eng.tensor_tensor(
                                out=d2[:], in0=dts[ah][:, pr_sl],
                                in1=dts[ah][:, pr_sl], op=ALU.mult)
                        # att-sum of d2 via PE: per 128-doc chunk a start/stop
                        # matmul pair -> nd_ps column, doc on partitions. Pairs
                        # stay back-to-back: PSUM allows only one pending
                        # accumulation group per region.
                        for c8 in range(8):
                            c = pp * 8 + c8
                            csl = slice(c8 * 128, (c8 + 1) * 128)
                            for ah in range(NAH):
                                nc.tensor.matmul(
                                    nd_ps[:, c:c + 1],
                                    lhsT=d2s[2 * pp + ah][:, csl],
                                    rhs=ones16[:, 0:1],
                                    start=(ah == 0), stop=(ah == 1))

                    # rd = exp(-0.5*ln(nd)) * dmask, doc on partitions
                    rd = wpool.tile([128, 16], f32, tag="rd")
                    nc.scalar.activation(rd[:], nd_ps[:], AF.Ln,
                                         bias=actb[:, 0:1])
                    nc.scalar.activation(rd[:], rd[:], AF.Exp, scale=-0.5)
                    nc.vector.tensor_tensor(
                        out=rd[:], in0=rd[:],
                        in1=dmask[:, b * NC16:(b + 1) * NC16], op=ALU.mult)

                    # simT: per doc-128-chunk, d^T chunk stationary, qhat moving
                    sp = ps.tile([128, 16 * QLEN], f32, tag="sps")
                    for c in range(NC16):
                        for ah in range(NAH):
                            nc.tensor.matmul(
                                sp[:, c * QLEN:(c + 1) * QLEN],
                                lhsT=dts[ah][:, c * 128:(c + 1) * 128],
                                rhs=qhat[:, ah, b * QLEN:(b + 1) * QLEN],
                                start=(ah == 0), stop=(ah == 1))
                    # copy to S with per-partition rd, reorder (c,q)->(q,c)
                    spv = sp[:].rearrange("p (c q) -> p c q", c=NC16)
                    rdb = rd[:].unsqueeze(2).broadcast_to([128, NC16, QLEN])
                    sout = Sv[:, rb, :, :].transpose((0, 2, 1))
                    nc.vector.tensor_tensor(out=sout, in0=spv, in1=rdb,
                                            op=ALU.mult)

                def rbf_work(base, bsize, S):
                    if "rbf" in ablate:
                        return
                    # Emits closures into `pending`; drained during next block.
                    nrq = bsize * QLEN
                    n = nrq * 16
                    red_ps = pk.tile([nrq, 16], f32, tag="red", name="red_ps")

                    def docsum(k, src):
                        def go():
                            sv = src[:].rearrange("p (rq c) -> p rq c", c=16)
                            for c in range(16):
                                nc.tensor.matmul(red_ps[:, k:k + 1],
                                                 lhsT=sv[:, :, c],
                                                 rhs=ones16[:, 0:1],
                                                 start=(c == 0), stop=(c == 15))
                        return go

                    s2 = rpool.tile([128, n], fp16, tag="s2", name="s2")
                    e0 = rpool.tile([128, n], bf16, tag="e0", name="e0")

                    def head():
                        nc.vector.tensor_tensor(out=s2[:], in0=S[:], in1=S[:],
                                                op=ALU.mult)
                        nc.scalar.activation(e0[:], s2[:], AF.Exp, scale=-50.0)
                    pending.append(head)

                    for k, mu in enumerate(MUS):
                        fk = rpool.tile([128, n], bf16, tag="fk", name="fk")
                        gk = rpool.tile([128, n], fp16, tag="gk", name="gk")

                        def mu_item(k=k, mu=mu, fk=fk, gk=gk):
                            nc.scalar.activation(fk[:], S[:], AF.Exp,
                                                 scale=100.0 * mu,
                                                 bias=actb[:, 3 + k:4 + k])
                            nc.vector.tensor_tensor(out=gk[:], in0=e0[:],
                                                    in1=fk[:], op=ALU.mult)
                            docsum(k, gk)()
                        pending.append(mu_item)

                    u = rpool.tile([128, n], fp16, tag="t2", name="u")
                    t2 = rpool.tile([128, n], fp16, tag="fk", name="t2")
                    e10 = rpool.tile([128, n], fp16, tag="e10", name="e10")

                    def sharp():
                        # sharp kernel (mu=1, sigma=0.001): u = s-1 and u*u
                        # on DVE (4x/2x modes), single Exp pass on Act
                        nc.vector.tensor_scalar(out=u[:], in0=S[:],
                                                scalar1=-1.0, scalar2=None,
                                                op0=ALU.add)
                        nc.vector.tensor_tensor(out=t2[:], in0=u[:],
                                                in1=u[:], op=ALU.mult)
                        nc.scalar.activation(e10[:], t2[:], AF.Exp,
                                             scale=-500000.0)
                        docsum(10, e10)()
                    pending.append(sharp)
                    pending.append(docsum(11, S))

                    def tail():
                        # lg = ln(red + 1e-6); w = (docsum != 0); dot W_c;
                        # q-sum via tiny matmul into the scores row.
                        eq = fpool.tile([nrq, 1], f32, tag="eq", name="eq")
                        nc.vector.tensor_scalar(out=eq[:], in0=red_ps[:, 11:12],
                                                scalar1=0.0, scalar2=None,
                                                op0=ALU.is_equal)
                        w = fpool.tile([nrq, 1], f32, tag="w", name="w")
                        nc.vector.tensor_scalar(out=w[:], in0=eq[:], scalar1=-1.0,
                                                scalar2=1.0, op0=ALU.mult,
                                                op1=ALU.add)
                        lg = fpool.tile([nrq, 11], fp16, tag="lg", name="lg")
                        nc.scalar.activation(lg[:], red_ps[:, 0:11], AF.Ln,
                                             bias=actb[0:nrq, 1:2])
                        p1 = fpool.tile([nrq, 11], fp16, tag="p1", name="p1")
                        nc.vector.tensor_tensor(out=p1[:], in0=lg[:],
                                                in1=wcb[0:nrq], op=ALU.mult)
                        dot = fpool.tile([nrq, 1], f32, tag="dot", name="dot")
                        nc.vector.reduce_sum(out=dot[:], in_=p1[:], axis=AX.X)
                        mdot = fpool.tile([nrq, 1], f32, tag="mdot", name="mdot")
                        nc.vector.tensor_tensor(out=mdot[:], in0=dot[:],
                                                in1=w[:], op=ALU.mult)
                        sc_ps = pr.tile([1, bsize], f32, tag="small",
                                        name="sc_ps")
                        nc.tensor.matmul(sc_ps[:], lhsT=mdot[:],
                                         rhs=bmask[0:nrq, 0:bsize],
                                         start=True, stop=True)
                        nc.vector.tensor_scalar(
                            out=scores[:, base:base + bsize],
                            in0=sc_ps[:], scalar1=bc, scalar2=None, op0=ALU.add)
                    pending.append(tail)

                base = 0
                for bi, bsize in enumerate(blocks):
                    S = rpool.tile([128, bsize * QLEN * 16], fp16, tag="S",
                                   name="S")
                    Sv = S[:].rearrange("p (r q c) -> p r q c", r=bsize, c=16)
                    per_batch = -(-len(pending) // bsize) if pending else 0
                    for rb in range(bsize):
                        batch_work(base + rb, rb, S, Sv)
                        drain(per_batch)
                    drain(len(pending))
                    rbf_work(base, bsize, S)
                    base += bsize
                drain(len(pending))

                nc.sync.dma_start(out=d_out.ap(), in_=scores[:])

    try:
        nc.compile()
    finally:
        hw_specs.get_activation_tables = _orig_tables
        _bacc_mod.get_activation_tables = _orig_tables
    return nc


# ---------------------------------------------------------------- host prep
def _prep_core(core, query_tok, doc_tok, query_entity, doc_entity,
               embed_table, W_t, b_t, W_e, b_e, W_c, b_c, pack=RBF_PACK):
    bs = slice(core * BPC, (core + 1) * BPC)
    qt = np.asarray(query_tok[bs], dtype=np.int64) + 1   # [8, 32]
    dt = np.asarray(doc_tok[bs], dtype=np.int64) + 1     # [8, 2048]

    uniq, dinv = np.unique(np.concatenate([dt.ravel(), qt.ravel()]),
                           return_inverse=True)
    nuniq = len(uniq)
    assert nuniq <= 17000
    tab = np.zeros((17000, WPAD), dtype=np.float16)
    tab[:nuniq, :WORD] = embed_table[uniq].astype(np.float16)
    didx = dinv[:BPC * DLEN].reshape(BPC, DLEN).astype(np.int16)
    qidx = dinv[BPC * DLEN:].reshape(BPC * QLEN).astype(np.int16)

    def wrap(a):  # [n] -> [128, n//16] wrapped in 16 partitions, replicated x8
        w = a.reshape(-1, 16).T  # [16, n/16]
        return np.tile(w, (8, 1)).copy()

    idxd = np.concatenate([wrap(didx[b]) for b in range(BPC)], axis=1)
    idxq = wrap(qidx)
    ci16 = np.ascontiguousarray(np.concatenate([idxd, idxq], axis=1))

    entT = np.ascontiguousarray(
        np.asarray(doc_entity[bs], dtype=np.float16).transpose(0, 2, 1))
    qentT = np.ascontiguousarray(
        np.asarray(query_entity[bs], dtype=np.float16).transpose(0, 2, 1)
    ).transpose(1, 0, 2).reshape(128, BPC * QLEN)

    wtp = np.zeros((WPAD, ATT), dtype=np.float16)
    wtp[:WORD] = W_t.astype(np.float16)
    wt = np.concatenate(
        [wtp[wc * 128:(wc + 1) * 128, ah * 128:(ah + 1) * 128]
         for wc in range(NWCH) for ah in range(NAH)], axis=1)
    we = W_e.astype(np.float16)
    wcb = np.tile(np.asarray(W_c, dtype=np.float16).reshape(1, 11), (128, 1))
    wcb = np.concatenate(
        [wcb, np.zeros((128, C16_TOT - C16_WC - 11), np.float16)], axis=1)
    cf16 = np.ascontiguousarray(np.concatenate(
        [wt, we, qentT, np.ones((128, 128), np.float16), wcb], axis=1))
    assert cf16.shape[1] == C16_TOT

    bias = (b_t + b_e).astype(np.float32).reshape(NAH, 128).T
    dmask = (dt != 0).astype(np.float32)  # [8, 2048]
    dmask = dmask.reshape(BPC, NC16, 128).transpose(2, 0, 1).reshape(
        128, BPC * NC16)
    bmask = np.zeros((128, 4), dtype=np.float32)
    for r in range(pack):
        bmask[r * QLEN:(r + 1) * QLEN, r] = 1.0
    actb = np.tile(np.array(
        [1e-30, 1e-6, -707.10678] + [-50.0 * m * m for m in MUS],
        dtype=np.float32), (128, 1))
    cf32 = np.ascontiguousarray(np.concatenate(
        [bias, dmask, bmask, actb], axis=1))
    assert cf32.shape[1] == C32_TOT

    qmask = (qt != 0).astype(np.float32).reshape(1, BPC * QLEN)
    qrow = np.zeros((1, 257), dtype=np.float32)
    qrow[0, :BPC * QLEN] = qmask
    qrow[0, 256] = np.asarray(b_c, dtype=np.float32).reshape(())

    return {
        "tab": tab, "ci16": ci16, "cf16": cf16, "cf32": cf32, "entT": entT,
        "qrow": qrow, "onesf": np.ones((1, 128), np.float16),
    }


def kernel(**inputs):
    from concourse import bass_utils

    if "nc" not in _CACHE:
        _CACHE["nc"] = _build_program()
    nc = _CACHE["nc"]

    args = {k: np.asarray(v) for k, v in inputs.items()}
    in_maps = [_prep_core(c, **args) for c in range(NCORES)]
    res = bass_utils.run_bass_kernel_spmd(nc, in_maps,
                                          core_ids=list(range(NCORES)))
    out = np.concatenate([res.results[c]["out"].reshape(BPC)
                          for c in range(NCORES)])
    return out.reshape(B, 1).astype(np.float32)



# revision 8
# speedup vs baseline: 126.8707x; 126.8707x over previous
"""KNRM-KG ranker kernel for 8 Trainium2 NeuronCores (Bass/Tile).

Strategy: pure data parallel over the batch dim (64 = 8 cores x 8 batches).
Device-side pipeline per batch:
  - transposed dma_gather of word embeddings from a per-core vocabulary-
    compacted fp16 table (rows padded to 384 cols) -> reps arrive with the
    word dim on partitions (no on-device transposes needed)
  - fp16 matmuls accumulate word + entity transforms in PSUM; bias+relu are
    fused into the PSUM->SBUF copy (tensor_scalar add+max on DVE for ah=0,
    Relu activation on Act for ah=1 - balances the two engines)
  - doc norms: d2 = dts^2 on DVE, then att-sum via PE (per 128-doc chunk an
    accumulating matmul pair lhsT=d2-chunk, rhs=ones column) -> nd lands in
    PSUM with doc on partitions; rd = exp(-0.5*ln(nd)) * dmask on Act/DVE
  - simT = d^T-chunk (stationary) @ qhat (moving) per doc chunk; the
    PSUM->SBUF copy applies rd per partition and reorders (c,q)->(q,c)
  - RBF bank: shared E0 = exp(-50 s^2), per-mu Fk = exp(100 mu s - 50 mu^2),
    Gk = E0*Fk on VectorE; sigma=0.001 kernel via Square+Exp; the doc-sum of
    each kernel runs on PE: 16 accumulating matmuls (lhsT = Gk chunk strided,
    rhs = ones column) -> PSUM column per kernel, (r,q) on partitions
  - tail per block: ln/W_c-dot/mask on [rq, 11] tiles, q-sum via one tiny
    matmul into the scores row
  - RBF work of block i is interleaved between the batches of block i+1
    (software pipelining; each engine executes its queue in order, so without
    interleaving PE idles during RBF and DVE/Act idle during batch phases)
"""

import numpy as np

# ---------------------------------------------------------------- constants
B, QLEN, DLEN = 64, 32, 2048
VOCAB, WORD, ENT, ATT = 50000, 300, 128, 256
NCORES = 8
BPC = B // NCORES            # batches per core
WPAD = 384                   # padded word dim (3 x 128, 768B fp16 rows)
NWCH = 3                     # word chunks of 128
NAH = 2                      # att halves of 128
NDC = DLEN // 512            # doc 512-chunks (4)
NC16 = DLEN // 128           # doc 128-chunks (16)
MUS = [-0.9, -0.7, -0.5, -0.3, -0.1, 0.1, 0.3, 0.5, 0.7, 0.9]  # sigma=0.1
RBF_PACK = 4                 # batches per RBF block
GSZ = 512                    # idxs per dma_gather (HW cap)

# packed fp16 constant block offsets
C16_WT, C16_WE, C16_QE, C16_ON, C16_WC = 0, 768, 1024, 1280, 1408
C16_TOT = 1424
# packed f32 constant block offsets
C32_BIAS, C32_DM, C32_BM, C32_ACT = 0, 2, 130, 134
C32_TOT = 147

_CACHE = {}


# ---------------------------------------------------------------- program
BLOCKS = [2, 2, 2, 2]


def _build_program(spk=True, pack=RBF_PACK, blocks=None, nreps=1,
                   ablate=(), gsz=GSZ, scratch=16384):
    import concourse.bacc as bacc
    import concourse.mybir as mybir
    import concourse.tile as tile

    fp16 = mybir.dt.float16
    f32 = mybir.dt.float32
    i16 = mybir.dt.int16
    bf16 = mybir.dt.bfloat16
    AF = mybir.ActivationFunctionType
    ALU = mybir.AluOpType
    AX = mybir.AxisListType

    if blocks is None:
        blocks = list(BLOCKS)
    assert sum(blocks) == BPC

    # Pin all activations to the one table set covering Exp/Ln/Square so the
    # act-table pass emits a single load instead of thrashing between sets.
    import concourse.hw_specs as hw_specs
    _orig_tables = hw_specs.get_activation_tables

    def _one_set(arch):
        t = _orig_tables(arch)
        return {k: (v if k == "natural_log_exp_and_others" else frozenset())
                for k, v in t.items()}

    hw_specs.get_activation_tables = _one_set
    import concourse.bacc as _bacc_mod
    _bacc_mod.get_activation_tables = _one_set
    nc = bacc.Bacc("TRN2", target_bir_lowering=False, debug=False,
                   num_devices=NCORES, dynamic_dma_scratch_size=scratch)

    # DRAM inputs (per-core)
    d_tab = nc.dram_tensor("tab", [17000, WPAD], fp16, kind="ExternalInput")
    d_ci16 = nc.dram_tensor("ci16", [128, BPC * 128 + 16], i16,
                            kind="ExternalInput")
    d_cf16 = nc.dram_tensor("cf16", [128, C16_TOT], fp16,
                            kind="ExternalInput")
    d_cf32 = nc.dram_tensor("cf32", [128, C32_TOT], f32,
                            kind="ExternalInput")
    d_entT = nc.dram_tensor("entT", [BPC, 128, DLEN], fp16,
                            kind="ExternalInput")
    d_qrow = nc.dram_tensor("qrow", [1, 257], f32, kind="ExternalInput")
    d_onesf = nc.dram_tensor("onesf", [1, 128], fp16, kind="ExternalInput")
    d_out = nc.dram_tensor("out", [1, BPC], f32, kind="ExternalOutput")

    with tile.TileContext(nc) as tc:
        with (
            tc.tile_pool(name="const", bufs=1) as cpool,
            tc.tile_pool(name="gath", bufs=2) as gpool,
            tc.tile_pool(name="ent", bufs=2) as epool,
            tc.tile_pool(name="dt", bufs=2) as dpool,
            tc.tile_pool(name="work", bufs=2) as wpool,
            tc.tile_pool(name="rbf", bufs=2) as rpool,
            tc.tile_pool(name="fin", bufs=2) as fpool,
            tc.tile_pool(name="pt", bufs=2, space="PSUM") as pt,   # transform
            tc.tile_pool(name="ps", bufs=1, space="PSUM") as ps,   # simT
            tc.tile_pool(name="pn", bufs=1, space="PSUM") as pn,   # norms
            tc.tile_pool(name="pr", bufs=1, space="PSUM") as pr,   # misc
            tc.tile_pool(name="pk", bufs=1, space="PSUM") as pk,   # rbf sums
        ):
            # ---------------- load constants (packed by dtype)
            ci16 = cpool.tile([128, BPC * 128 + 16], i16)
            nc.sync.dma_start(out=ci16[:], in_=d_ci16.ap())
            cf16 = cpool.tile([128, C16_TOT], fp16)
            nc.sync.dma_start(out=cf16[:], in_=d_cf16.ap())
            cf32 = cpool.tile([128, C32_TOT], f32)
            nc.sync.dma_start(out=cf32[:], in_=d_cf32.ap())
            qrow = cpool.tile([1, 257], f32)
            nc.sync.dma_start(out=qrow[:], in_=d_qrow.ap())
            onesf = cpool.tile([1, 128], fp16)
            nc.sync.dma_start(out=onesf[:], in_=d_onesf.ap())

            idxd = ci16[:, 0:BPC * 128]
            idxq = ci16[:, BPC * 128:BPC * 128 + 16]
            we = cf16[:, C16_WE:C16_WE + ATT]
            qentT = cf16[:, C16_QE:C16_QE + BPC * QLEN]
            ones16 = cf16[:, C16_ON:C16_ON + 128]
            wcb = cf16[:, C16_WC:C16_WC + 11]
            bias = cf32[:, C32_BIAS:C32_BIAS + NAH]
            dmask = cf32[:, C32_DM:C32_DM + BPC * NC16]
            bmask = cf32[:, C32_BM:C32_BM + 4]
            actb = cf32[:, C32_ACT:C32_ACT + 13]
            qmask = qrow[:, 0:BPC * QLEN]
            bc = qrow[:, 256:257]

            def wt_l(wc, ah):  # W_t lhsT block [128 word, 128 att]
                o = C16_WT + (wc * NAH + ah) * 128
                return cf16[:, o:o + 128]
            for _rep in range(nreps):

                # ---------------- query side (once per core)
                qg = cpool.tile([128, NWCH, BPC * QLEN], fp16)
                if "qg" in ablate:
                    nc.gpsimd.memset(qg[:], 0.01)
                else:
                    nc.gpsimd.dma_gather(
                        out_ap=qg[:], in_ap=d_tab.ap(), idxs_ap=idxq,
                        num_idxs=BPC * QLEN, num_idxs_reg=BPC * QLEN,
                        elem_size=WPAD, transpose=True, single_packet=spk)

                qhat = cpool.tile([128, NAH, BPC * QLEN], fp16)
                q2 = wpool.tile([128, BPC * QLEN], fp16, tag="q2")
                nq_ps = pr.tile([1, BPC * QLEN], f32, tag="small")
                for ah in range(NAH):
                    qp = pt.tile([128, 512], f32, tag="tps")
                    for wc in range(NWCH):
                        nc.tensor.matmul(qp[:, :BPC * QLEN], lhsT=wt_l(wc, ah),
                                         rhs=qg[:, wc, :], start=(wc == 0),
                                         stop=False)
                    nc.tensor.matmul(qp[:, :BPC * QLEN],
                                     lhsT=we[:, ah * 128:(ah + 1) * 128],
                                     rhs=qentT, start=False, stop=True)
                    # bias + relu fused
                    nc.vector.tensor_scalar(
                        out=qhat[:, ah, :], in0=qp[:, :BPC * QLEN],
                        scalar1=bias[:, ah:ah + 1], scalar2=0.0,
                        op0=ALU.add, op1=ALU.max)
                    nc.vector.tensor_tensor(out=q2[:], in0=qhat[:, ah, :],
                                            in1=qhat[:, ah, :], op=ALU.mult)
                    nc.tensor.matmul(nq_ps[:], lhsT=ones16[:, 0:1], rhs=q2[:],
                                     start=(ah == 0), stop=(ah == 1))
                # rq = exp(-0.5*ln(nq)) * qmask  (single partition, 256 elems)
                rq = cpool.tile([1, BPC * QLEN], f32)
                nc.scalar.activation(rq[:], nq_ps[:], AF.Ln, bias=actb[0:1, 0:1])
                nc.scalar.activation(rq[:], rq[:], AF.Exp, scale=-0.5)
                nc.vector.tensor_tensor(out=rq[:], in0=rq[:], in1=qmask,
                                        op=ALU.mult)
                rq16 = cpool.tile([1, BPC * QLEN], fp16)
                nc.vector.tensor_copy(rq16[:], rq[:])
                rqb_ps = pr.tile([128, BPC * QLEN], f32, tag="small")
                nc.tensor.matmul(rqb_ps[:], lhsT=onesf[:], rhs=rq16[:],
                                 start=True, stop=True)
                for ah in range(NAH):
                    nc.vector.tensor_tensor(out=qhat[:, ah, :], in0=qhat[:, ah, :],
                                            in1=rqb_ps[:], op=ALU.mult)

                # ---------------- main loop: batches + pipelined RBF
                scores = cpool.tile([1, BPC], f32)
                if "rbf" in ablate or "docsum" in ablate:
                    nc.gpsimd.memset(scores[:], 0.0)
                pending = []       # RBF closures of the previous block

                abl = {}

                def abl_tile(tag, shape, dt):
                    # one-time garbage tile for ablation timing variants
                    if tag not in abl:
                        t = cpool.tile(shape, dt, name=f"abl_{tag}")
                        nc.gpsimd.memset(t[:], 0.01)
                        abl[tag] = t
                    return abl[tag]

                def drain(k):
                    for _ in range(min(k, len(pending))):
                        pending.pop(0)()

                ents = {}

                def fetch_ent(b):
                    # prefetch entity reps one batch ahead so the DMA
                    # latency hides under the previous batch's transform
                    if b < BPC and b not in ents:
                        t = epool.tile([128, DLEN], fp16, tag="ent",
                                       name="ent")
                        nc.sync.dma_start(out=t[:], in_=d_entT.ap()[b])
                        ents[b] = t

                def batch_work(b, rb, S, Sv):
                    # gather doc reps (transposed), GSZ idxs per instruction
                    gs = []
                    for gh in range(DLEN // gsz):
                        if "dg" in ablate:
                            gs.append(abl_tile("g", [128, NWCH, gsz], fp16))
                            continue
                        g = gpool.tile([128, NWCH, gsz], fp16,
                                       tag=f"g{gh}", name=f"g{gh}")
                        o = b * 128 + gh * (gsz // 16)
                        nc.gpsimd.dma_gather(
                            out_ap=g[:], in_ap=d_tab.ap(),
                            idxs_ap=idxd[:, o:o + gsz // 16],
                            num_idxs=gsz, num_idxs_reg=gsz,
                            elem_size=WPAD, transpose=True,
                            single_packet=spk)
                        gs.append(g)
                    fetch_ent(b)
                    ent = ents.pop(b)
                    fetch_ent(b + 1)

                    if "tf" in ablate:
                        dts = [abl_tile(f"dt{ah}", [128, DLEN], fp16)
                               for ah in range(NAH)]
                    else:
                        dts = [dpool.tile([128, DLEN], fp16, tag=f"dt{ah}",
                                          name=f"dt{ah}") for ah in range(NAH)]
                    nd_ps = pn.tile([128, NC16], f32, tag="ndps", name="nd_ps")
                    d2s = []
                    for pp in range(NDC // 2):
                        dcs = (2 * pp, 2 * pp + 1)
                        sls = [slice(dc * 512, (dc + 1) * 512) for dc in dcs]
                        pr_sl = slice(sls[0].start, sls[1].stop)
                        for ah in range(NAH):
                            # one 2-bank PSUM tile per pp; matmuls target the
                            # in-bank halves, bias+relu reads span both banks
                            tps = pt.tile([128, 1024], f32, tag="tps",
                                          name="tps")
                            if "tf" not in ablate:
                                for wc in range(NWCH):
                                    for j, dc in enumerate(dcs):
                                        gv = gs[dc * 512 // gsz]
                                        go = dc * 512 % gsz
                                        nc.tensor.matmul(
                                            tps[:, j * 512:(j + 1) * 512],
                                            lhsT=wt_l(wc, ah),
                                            rhs=gv[:, wc, go:go + 512],
                                            start=(wc == 0), stop=False)
                                for j, dc in enumerate(dcs):
                                    nc.tensor.matmul(
                                        tps[:, j * 512:(j + 1) * 512],
                                        lhsT=we[:, ah * 128:(ah + 1) * 128],
                                        rhs=ent[:, sls[j]], start=False,
                                        stop=True)
                                if ah == 0:
                                    nc.vector.tensor_scalar(
                                        out=dts[ah][:, pr_sl], in0=tps[:],
                                        scalar1=bias[:, ah:ah + 1],
                                        scalar2=0.0, op0=ALU.add, op1=ALU.max)
                                else:
                                    nc.scalar.activation(
                                        dts[ah][:, pr_sl], tps[:],
                                        AF.Relu, bias=bias[:, ah:ah + 1])
                            d2 = wpool.tile([128, 1024], fp16, tag="d2",
                                            name="d2")
                            d2s.append(d2)
                            if "norm" not in ablate:
                                d2_eng = (nc.gpsimd if (ah == 1 and pp == 0)
                                          else nc.vector)
                                d2_eng.tensor_tensor(
                                    out=d2[:], in0=dts[ah][:, pr_sl],
                                    in1=dts[ah][:, pr_sl], op=ALU.mult)
                        # att-sum of d2 via PE: per 128-doc chunk a start/stop
                        # matmul pair -> nd_ps column, doc on partitions. Pairs
                        # stay back-to-back: PSUM allows only one pending
                        # accumulation group per region.
                        if "norm" not in ablate:
                            for c8 in range(8):
                                c = pp * 8 + c8
                                csl = slice(c8 * 128, (c8 + 1) * 128)
                                for ah in range(NAH):
                                    nc.tensor.matmul(
                                        nd_ps[:, c:c + 1],
                                        lhsT=d2s[2 * pp + ah][:, csl],
                                        rhs=ones16[:, 0:1],
                                        start=(ah == 0), stop=(ah == 1))

                    # rd = exp(-0.5*ln(nd)) * dmask, doc on partitions
                    if "norm" in ablate:
                        rd = abl_tile("rd", [128, 16], f32)
                    else:
                        rd = wpool.tile([128, 16], f32, tag="rd")
                        nc.scalar.activation(rd[:], nd_ps[:], AF.Ln,
                                             bias=actb[:, 0:1])
                        nc.scalar.activation(rd[:], rd[:], AF.Exp, scale=-0.5)
                        nc.vector.tensor_tensor(
                            out=rd[:], in0=rd[:],
                            in1=dmask[:, b * NC16:(b + 1) * NC16], op=ALU.mult)

                    # simT: per doc-128-chunk, d^T chunk stationary, qhat moving
                    sp = ps.tile([128, 16 * QLEN], f32, tag="sps")
                    if "sim" not in ablate:
                        for c in range(NC16):
                            for ah in range(NAH):
                                nc.tensor.matmul(
                                    sp[:, c * QLEN:(c + 1) * QLEN],
                                    lhsT=dts[ah][:, c * 128:(c + 1) * 128],
                                    rhs=qhat[:, ah, b * QLEN:(b + 1) * QLEN],
                                    start=(ah == 0), stop=(ah == 1))
                        # copy to S with per-partition rd, reorder (c,q)->(q,c)
                        spv = sp[:].rearrange("p (c q) -> p c q", c=NC16)
                        rdb = rd[:].unsqueeze(2).broadcast_to([128, NC16, QLEN])
                        sout = Sv[:, rb, :, :].transpose((0, 2, 1))
                        nc.vector.tensor_tensor(out=sout, in0=spv, in1=rdb,
                                                op=ALU.mult)

                def rbf_work(base, bsize, S):
                    if "rbf" in ablate:
                        return
                    # Emits closures into `pending`; drained during next block.
                    nrq = bsize * QLEN
                    n = nrq * 16
                    red_ps = pk.tile([nrq, 16], f32, tag="red", name="red_ps")

                    def docsum(k, src):
                        def go():
                            sv = src[:].rearrange("p (rq c) -> p rq c", c=16)
                            for c in range(16):
                                nc.tensor.matmul(red_ps[:, k:k + 1],
                                                 lhsT=sv[:, :, c],
                                                 rhs=ones16[:, 0:1],
                                                 start=(c == 0), stop=(c == 15))
                        return go

                    s2 = rpool.tile([128, n], fp16, tag="s2", name="s2")
                    e0 = rpool.tile([128, n], bf16, tag="e0", name="e0")

                    def head():
                        nc.vector.tensor_tensor(out=s2[:], in0=S[:], in1=S[:],
                                                op=ALU.mult)
                        nc.scalar.activation(e0[:], s2[:], AF.Exp, scale=-50.0)
                    pending.append(head)

                    for k, mu in enumerate(MUS):
                        fk = rpool.tile([128, n], bf16, tag="fk", name="fk")
                        gk = rpool.tile([128, n], fp16, tag="gk", name="gk")

                        def mu_item(k=k, mu=mu, fk=fk, gk=gk):
                            nc.scalar.activation(fk[:], S[:], AF.Exp,
                                                 scale=100.0 * mu,
                                                 bias=actb[:, 3 + k:4 + k])
                            nc.vector.tensor_tensor(out=gk[:], in0=e0[:],
                                                    in1=fk[:], op=ALU.mult)
                            docsum(k, gk)()
                        pending.append(mu_item)

                    u = rpool.tile([128, n], fp16, tag="t2", name="u")
                    t2 = rpool.tile([128, n], fp16, tag="fk", name="t2")
                    e10 = rpool.tile([128, n], fp16, tag="e10", name="e10")

                    def sharp():
                        # sharp kernel (mu=1, sigma=0.001): u = s-1 and u*u
                        # on DVE (4x/2x modes), single Exp pass on Act
                        nc.vector.tensor_scalar(out=u[:], in0=S[:],
                                                scalar1=-1.0, scalar2=None,
                                                op0=ALU.add)
                        nc.vector.tensor_tensor(out=t2[:], in0=u[:],
                                                in1=u[:], op=ALU.mult)
                        nc.scalar.activation(e10[:], t2[:], AF.Exp,
                                             scale=-500000.0)
                        docsum(10, e10)()
                    pending.append(sharp)
                    pending.append(docsum(11, S))

                    def tail():
                        # lg = ln(red + 1e-6); w = (docsum != 0); dot W_c;
                        # q-sum via tiny matmul into the scores row.
                        eq = fpool.tile([nrq, 1], f32, tag="eq", name="eq")
                        nc.vector.tensor_scalar(out=eq[:], in0=red_ps[:, 11:12],
                                                scalar1=0.0, scalar2=None,
                                                op0=ALU.is_equal)
                        w = fpool.tile([nrq, 1], f32, tag="w", name="w")
                        nc.vector.tensor_scalar(out=w[:], in0=eq[:], scalar1=-1.0,
                                                scalar2=1.0, op0=ALU.mult,
                                                op1=ALU.add)
                        lg = fpool.tile([nrq, 11], fp16, tag="lg", name="lg")
                        nc.scalar.activation(lg[:], red_ps[:, 0:11], AF.Ln,
                                             bias=actb[0:nrq, 1:2])
                        p1 = fpool.tile([nrq, 11], fp16, tag="p1", name="p1")
                        nc.vector.tensor_tensor(out=p1[:], in0=lg[:],
                                                in1=wcb[0:nrq], op=ALU.mult)
                        dot = fpool.tile([nrq, 1], f32, tag="dot", name="dot")
                        nc.vector.reduce_sum(out=dot[:], in_=p1[:], axis=AX.X)
                        mdot = fpool.tile([nrq, 1], f32, tag="mdot", name="mdot")
                        nc.vector.tensor_tensor(out=mdot[:], in0=dot[:],
                                                in1=w[:], op=ALU.mult)
                        sc_ps = pr.tile([1, bsize], f32, tag="small",
                                        name="sc_ps")
                        nc.tensor.matmul(sc_ps[:], lhsT=mdot[:],
                                         rhs=bmask[0:nrq, 0:bsize],
                                         start=True, stop=True)
                        nc.vector.tensor_scalar(
                            out=scores[:, base:base + bsize],
                            in0=sc_ps[:], scalar1=bc, scalar2=None, op0=ALU.add)
                    pending.append(tail)

                base = 0
                for bi, bsize in enumerate(blocks):
                    if "sim" in ablate:
                        S = abl_tile("S", [128, bsize * QLEN * 16], fp16)
                    else:
                        S = rpool.tile([128, bsize * QLEN * 16], fp16,
                                       tag="S", name="S")
                    Sv = S[:].rearrange("p (r q c) -> p r q c", r=bsize, c=16)
                    per_batch = -(-len(pending) // bsize) if pending else 0
                    for rb in range(bsize):
                        batch_work(base + rb, rb, S, Sv)
                        drain(per_batch)
                    drain(len(pending))
                    rbf_work(base, bsize, S)
                    base += bsize
                drain(len(pending))

                nc.sync.dma_start(out=d_out.ap(), in_=scores[:])

    try:
        nc.compile()
    finally:
        hw_specs.get_activation_tables = _orig_tables
        _bacc_mod.get_activation_tables = _orig_tables
    return nc


# ---------------------------------------------------------------- host prep
def _prep_core(core, query_tok, doc_tok, query_entity, doc_entity,
               embed_table, W_t, b_t, W_e, b_e, W_c, b_c, pack=RBF_PACK):
    bs = slice(core * BPC, (core + 1) * BPC)
    qt = np.asarray(query_tok[bs], dtype=np.int64) + 1   # [8, 32]
    dt = np.asarray(doc_tok[bs], dtype=np.int64) + 1     # [8, 2048]

    uniq, dinv = np.unique(np.concatenate([dt.ravel(), qt.ravel()]),
                           return_inverse=True)
    nuniq = len(uniq)
    assert nuniq <= 17000
    tab = np.zeros((17000, WPAD), dtype=np.float16)
    tab[:nuniq, :WORD] = embed_table[uniq].astype(np.float16)
    didx = dinv[:BPC * DLEN].reshape(BPC, DLEN).astype(np.int16)
    qidx = dinv[BPC * DLEN:].reshape(BPC * QLEN).astype(np.int16)

    def wrap(a):  # [n] -> [128, n//16] wrapped in 16 partitions, replicated x8
        w = a.reshape(-1, 16).T  # [16, n/16]
        return np.tile(w, (8, 1)).copy()

    idxd = np.concatenate([wrap(didx[b]) for b in range(BPC)], axis=1)
    idxq = wrap(qidx)
    ci16 = np.ascontiguousarray(np.concatenate([idxd, idxq], axis=1))

    entT = np.ascontiguousarray(
        np.asarray(doc_entity[bs], dtype=np.float16).transpose(0, 2, 1))
    qentT = np.ascontiguousarray(
        np.asarray(query_entity[bs], dtype=np.float16).transpose(0, 2, 1)
    ).transpose(1, 0, 2).reshape(128, BPC * QLEN)

    wtp = np.zeros((WPAD, ATT), dtype=np.float16)
    wtp[:WORD] = W_t.astype(np.float16)
    wt = np.concatenate(
        [wtp[wc * 128:(wc + 1) * 128, ah * 128:(ah + 1) * 128]
         for wc in range(NWCH) for ah in range(NAH)], axis=1)
    we = W_e.astype(np.float16)
    wcb = np.tile(np.asarray(W_c, dtype=np.float16).reshape(1, 11), (128, 1))
    wcb = np.concatenate(
        [wcb, np.zeros((128, C16_TOT - C16_WC - 11), np.float16)], axis=1)
    cf16 = np.ascontiguousarray(np.concatenate(
        [wt, we, qentT, np.ones((128, 128), np.float16), wcb], axis=1))
    assert cf16.shape[1] == C16_TOT

    bias = (b_t + b_e).astype(np.float32).reshape(NAH, 128).T
    dmask = (dt != 0).astype(np.float32)  # [8, 2048]
    dmask = dmask.reshape(BPC, NC16, 128).transpose(2, 0, 1).reshape(
        128, BPC * NC16)
    bmask = np.zeros((128, 4), dtype=np.float32)
    for r in range(pack):
        bmask[r * QLEN:(r + 1) * QLEN, r] = 1.0
    actb = np.tile(np.array(
        [1e-30, 1e-6, -707.10678] + [-50.0 * m * m for m in MUS],
        dtype=np.float32), (128, 1))
    cf32 = np.ascontiguousarray(np.concatenate(
        [bias, dmask, bmask, actb], axis=1))
    assert cf32.shape[1] == C32_TOT

    qmask = (qt != 0).astype(np.float32).reshape(1, BPC * QLEN)
    qrow = np.zeros((1, 257), dtype=np.float32)
    qrow[0, :BPC * QLEN] = qmask
    qrow[0, 256] = np.asarray(b_c, dtype=np.float32).reshape(())

    return {
        "tab": tab, "ci16": ci16, "cf16": cf16, "cf32": cf32, "entT": entT,
        "qrow": qrow, "onesf": np.ones((1, 128), np.float16),
    }


def kernel(**inputs):
    from concourse import bass_utils

    if "nc" not in _CACHE:
        _CACHE["nc"] = _build_program()
    nc = _CACHE["nc"]

    args = {k: np.asarray(v) for k, v in inputs.items()}
    in_maps = [_prep_core(c, **args) for c in range(NCORES)]
    res = bass_utils.run_bass_kernel_spmd(nc, in_maps,
                                          core_ids=list(range(NCORES)))
    out = np.concatenate([res.results[c]["out"].reshape(BPC)
                          for c in range(NCORES)])
    return out.reshape(B, 1).astype(np.float32)



# revision 9
# speedup vs baseline: 141.4790x; 1.1151x over previous
"""KNRM-KG ranker kernel v2 for 8 Trainium2 NeuronCores (Bass/Tile).

Strategy: pure data parallel over batch (64 = 8 cores x 8). Major changes
vs v1:
  - embedding table pre-projected on host: tab = embed @ W_t + (b_t+b_e),
    so gathered rows are already in ATT space (256 fp16 cols = 512B rows,
    -33%% gather traffic) and the word-side matmuls disappear.
  - per-batch transform: PSUM accumulates I128 @ g (gather add) + W_e @ ent,
    relu fused in the PSUM->SBUF copy (split across Act/DVE).
  - sim layout [rq = 4 batches x 32 q on partitions, 2048 docs free]:
    per-batch matmuls lhsT=qhat-slice into shared per-block PSUM chunks.
  - doc norms: d2 squares on DVE, partition-sum via csel ones-matmuls into
    a [4,512] PSUM tile per batch (chunk c lands on partition c), one Act
    Ln pass -> blk[16,512] per block; pad-bias +1e4 folded in so
    exp underflows to exact 0 for padded docs; rdb = broadcast rd to rq
    partitions via sel-matmul + Act Exp; S = sim * rdb in the PSUM copy.
  - RBF docsums ride along the exp/multiply passes via accum_out /
    tensor_tensor_reduce -> no PE docsum matmuls at all.
  - software pipelining: RBF closures of block i drain during block i+1;
    pending carries across reps; out-DMA is the last closure per rep.
"""

import numpy as np

# ---------------------------------------------------------------- constants
B, QLEN, DLEN = 64, 32, 2048
VOCAB, WORD, ENT, ATT = 50000, 300, 128, 256
NCORES = 8
BPC = B // NCORES            # batches per core
NAH = 2                      # att halves of 128
NCH = 4                      # doc 512-chunks
MUS = [-0.9, -0.7, -0.5, -0.3, -0.1, 0.1, 0.3, 0.5, 0.7, 0.9]  # sigma=0.1
PACK = 4                     # batches per RBF block
GSZ = 512                    # idxs per dma_gather (HW cap)

# packed fp16 constant block offsets
C16_WE, C16_I, C16_QE, C16_ON, C16_CSEL, C16_SEL, C16_PB, C16_WC = (
    0, 256, 384, 640, 768, 784, 1296, 2320)
C16_TOT = 2332
# packed f32 constant block offsets (bmask, actb)
C32_BM, C32_ACT = 0, 4
C32_TOT = 27

_CACHE = {}


# ---------------------------------------------------------------- program
def _build_program(spk=True, nreps=1, ablate=(), gsz=GSZ, scratch=32768,
                   relu_split=0, sq_split=0, red_split=0, ent_eng="sync",
                   nqueues=2, red2=True, direct_k=7, dsq=3):
    import concourse.bacc as bacc
    import concourse.mybir as mybir
    import concourse.tile as tile

    fp16 = mybir.dt.float16
    f32 = mybir.dt.float32
    i16 = mybir.dt.int16
    bf16 = mybir.dt.bfloat16
    AF = mybir.ActivationFunctionType
    ALU = mybir.AluOpType
    AX = mybir.AxisListType

    import concourse.hw_specs as hw_specs
    _orig_tables = hw_specs.get_activation_tables

    def _one_set(arch):
        t = _orig_tables(arch)
        return {k: (v if k == "natural_log_exp_and_others" else frozenset())
                for k, v in t.items()}

    hw_specs.get_activation_tables = _one_set
    import concourse.bacc as _bacc_mod
    _bacc_mod.get_activation_tables = _one_set
    nc = bacc.Bacc("TRN2", target_bir_lowering=False, debug=False,
                   num_devices=NCORES, dynamic_dma_scratch_size=scratch,
                   num_swdge_queues=nqueues)

    # DRAM inputs (per-core)
    d_tab = nc.dram_tensor("tab", [17000, ATT], fp16, kind="ExternalInput")
    d_ci16 = nc.dram_tensor("ci16", [128, BPC * 128 + 16], i16,
                            kind="ExternalInput")
    d_cf16 = nc.dram_tensor("cf16", [128, C16_TOT], fp16,
                            kind="ExternalInput")
    d_cf32 = nc.dram_tensor("cf32", [128, C32_TOT], f32,
                            kind="ExternalInput")
    d_entT = nc.dram_tensor("entT", [BPC, 128, DLEN], fp16,
                            kind="ExternalInput")
    d_qrow = nc.dram_tensor("qrow", [1, 257], f32, kind="ExternalInput")
    d_onesf = nc.dram_tensor("onesf", [1, 128], fp16, kind="ExternalInput")
    d_out = nc.dram_tensor("out", [1, BPC], f32, kind="ExternalOutput")

    with tile.TileContext(nc) as tc:
        with (
            tc.tile_pool(name="const", bufs=1) as cpool,
            tc.tile_pool(name="gath", bufs=2) as gpool,
            tc.tile_pool(name="ent", bufs=2) as epool,
            tc.tile_pool(name="dt", bufs=2) as dpool,
            tc.tile_pool(name="work", bufs=2) as wpool,
            tc.tile_pool(name="rbf", bufs=2) as rpool,
            tc.tile_pool(name="fin", bufs=2) as fpool,
            tc.tile_pool(name="spool", bufs=2) as spool,
            tc.tile_pool(name="psim", bufs=1, space="PSUM") as psim,
            tc.tile_pool(name="ptps", bufs=2, space="PSUM") as ptps,
            tc.tile_pool(name="pnd", bufs=1, space="PSUM") as pnd,
            tc.tile_pool(name="prdb", bufs=1, space="PSUM") as prdb,
        ):
            # ---------------- load constants (packed by dtype)
            ci16 = cpool.tile([128, BPC * 128 + 16], i16)
            nc.sync.dma_start(out=ci16[:], in_=d_ci16.ap())
            cf16 = cpool.tile([128, C16_TOT], fp16)
            nc.sync.dma_start(out=cf16[:], in_=d_cf16.ap())
            cf32 = cpool.tile([128, C32_TOT], f32)
            nc.sync.dma_start(out=cf32[:], in_=d_cf32.ap())
            qrow = cpool.tile([1, 257], f32)
            nc.sync.dma_start(out=qrow[:], in_=d_qrow.ap())
            onesf = cpool.tile([1, 128], fp16)
            nc.sync.dma_start(out=onesf[:], in_=d_onesf.ap())

            idxd = ci16[:, 0:BPC * 128]
            idxq = ci16[:, BPC * 128:BPC * 128 + 16]
            we = cf16[:, C16_WE:C16_WE + ATT]
            iden = cf16[:, C16_I:C16_I + 128]
            qentT = cf16[:, C16_QE:C16_QE + BPC * QLEN]
            onescol = cf16[:, C16_ON:C16_ON + 128]
            cselv = cf16[:, C16_CSEL:C16_CSEL + 16].rearrange(
                "p (c m) -> p c m", c=4)
            selv = cf16[:, C16_SEL:C16_SEL + 512].rearrange(
                "p (c m) -> p c m", c=4)
            pbv = cf16[:, C16_PB:C16_PB + 1024].rearrange(
                "p (k j) -> p k j", k=2)  # [128, 2, 512]
            wcb = cf16[:, C16_WC:C16_WC + 11]
            bmask = cf32[:, C32_BM:C32_BM + 4]
            actb = cf32[:, C32_ACT:C32_ACT + 23]
            qmask = qrow[:, 0:BPC * QLEN]
            bc = qrow[:, 256:257]

            pending = []

            def drain(k):
                for _ in range(min(k, len(pending))):
                    pending.pop(0)()

            for _rep in range(nreps):
                # ---------------- query side (once per rep)
                qg = cpool.tile([128, NAH, BPC * QLEN], fp16, tag="qg")
                if "qg" in ablate:
                    nc.gpsimd.memset(qg[:], 0.01)
                else:
                    nc.gpsimd.dma_gather(
                        out_ap=qg[:], in_ap=d_tab.ap(), idxs_ap=idxq,
                        num_idxs=BPC * QLEN, num_idxs_reg=BPC * QLEN,
                        elem_size=ATT, transpose=True, single_packet=spk)

                qhat = cpool.tile([128, NAH, BPC * QLEN], fp16, tag="qhat")
                q2 = wpool.tile([128, BPC * QLEN], fp16, tag="q2")
                nq_ps = pnd.tile([1, BPC * QLEN], f32, tag="ndp")
                for ah in range(NAH):
                    qp = ptps.tile([128, BPC * QLEN], f32, tag="tps")
                    nc.tensor.matmul(qp[:], lhsT=iden, rhs=qg[:, ah, :],
                                     start=True, stop=False)
                    nc.tensor.matmul(qp[:], lhsT=we[:, ah * 128:(ah + 1) * 128],
                                     rhs=qentT, start=False, stop=True)
                    nc.vector.tensor_scalar(
                        out=qhat[:, ah, :], in0=qp[:], scalar1=0.0,
                        scalar2=None, op0=ALU.max)
                    nc.vector.tensor_tensor(out=q2[:], in0=qhat[:, ah, :],
                                            in1=qhat[:, ah, :], op=ALU.mult)
                    nc.tensor.matmul(nq_ps[:], lhsT=onescol[:, 0:1], rhs=q2[:],
                                     start=(ah == 0), stop=(ah == 1))
                rq = cpool.tile([1, BPC * QLEN], f32, tag="rq")
                nc.scalar.activation(rq[:], nq_ps[:], AF.Ln, bias=actb[0:1, 0:1])
                nc.scalar.activation(rq[:], rq[:], AF.Exp, scale=-0.5)
                nc.vector.tensor_tensor(out=rq[:], in0=rq[:], in1=qmask,
                                        op=ALU.mult)
                rq16 = cpool.tile([1, BPC * QLEN], fp16, tag="rq16")
                nc.vector.tensor_copy(rq16[:], rq[:])
                rqb_ps = prdb.tile([128, BPC * QLEN], f32, tag="rdb")
                nc.tensor.matmul(rqb_ps[:], lhsT=onesf[:], rhs=rq16[:],
                                 start=True, stop=True)
                for ah in range(NAH):
                    nc.vector.tensor_tensor(out=qhat[:, ah, :],
                                            in0=qhat[:, ah, :],
                                            in1=rqb_ps[:], op=ALU.mult)

                scores = spool.tile([1, BPC], f32, tag="scores")
                if "rbf" in ablate:
                    nc.gpsimd.memset(scores[:], 0.0)

                ents = {}

                ent_q = {"sync": nc.sync, "scalar": nc.scalar,
                         "vector": nc.vector}[ent_eng]

                def fetch_ent(b):
                    if b < BPC and b not in ents:
                        t = epool.tile([128, DLEN], fp16, tag="ent",
                                       name="ent")
                        ent_q.dma_start(out=t[:], in_=d_entT.ap()[b])
                        ents[b] = t

                # state shared between batch closures
                state = {}

                def batch_work(b, rb, S, blk, simcs):
                    # ---- gathers: DLEN idxs in gsz-sized chunks
                    gs = []
                    for gh in range(DLEN // gsz):
                        g = gpool.tile([128, NAH, gsz], fp16,
                                       tag=f"g{gh}", name=f"g{gh}")
                        o = b * 128 + gh * (gsz // 16)
                        if "dg" in ablate:
                            nc.gpsimd.memset(g[:], 0.01)
                        else:
                            nc.gpsimd.dma_gather(
                                out_ap=g[:], in_ap=d_tab.ap(),
                                idxs_ap=idxd[:, o:o + gsz // 16],
                                num_idxs=gsz, num_idxs_reg=gsz,
                                elem_size=ATT, transpose=True,
                                single_packet=spk, queue_num=gh % nqueues)
                        gs.append(g)
                    fetch_ent(b)
                    ent = ents.pop(b)
                    fetch_ent(b + 1)

                    # ---- transform: PSUM = I@g + We@ent; relu in the copy
                    dts = [dpool.tile([128, DLEN], fp16, tag=f"dt{ah}",
                                      name=f"dt{ah}") for ah in range(NAH)]
                    for ah in range(NAH):
                        for c in range(NCH):
                            sl = slice(c * 512, (c + 1) * 512)
                            gv = gs[(c * 512) // gsz]
                            go = (c * 512) % gsz
                            tps = ptps.tile([128, 512], f32, tag="tps",
                                            name="tps")
                            nc.tensor.matmul(tps[:], lhsT=iden,
                                             rhs=gv[:, ah, go:go + 512],
                                             start=True, stop=False)
                            nc.tensor.matmul(
                                tps[:], lhsT=we[:, ah * 128:(ah + 1) * 128],
                                rhs=ent[:, sl], start=False, stop=True)
                            if (ah * NCH + c) % 2 < relu_split:
                                nc.scalar.activation(dts[ah][:, sl], tps[:],
                                                     AF.Relu)
                            else:
                                nc.vector.tensor_scalar(
                                    out=dts[ah][:, sl], in0=tps[:],
                                    scalar1=0.0, scalar2=None, op0=ALU.max)

                    # ---- norms: d2 squares, csel ones-matmuls -> ndp[4,512]
                    d2s = []
                    for ah in range(NAH):
                        d2 = wpool.tile([128, DLEN], fp16, tag=f"d2{ah}",
                                        name=f"d2{ah}")
                        if ah < sq_split:
                            nc.scalar.activation(d2[:], dts[ah][:], AF.Square)
                        else:
                            nc.vector.tensor_tensor(out=d2[:], in0=dts[ah][:],
                                                    in1=dts[ah][:],
                                                    op=ALU.mult)
                        d2s.append(d2)
                    ndp = pnd.tile([4, 512], f32, tag="ndp", name="ndp")
                    for c in range(NCH):
                        sl = slice(c * 512, (c + 1) * 512)
                        for ah in range(NAH):
                            nc.tensor.matmul(
                                ndp[:], lhsT=cselv[:, c, :], rhs=d2s[ah][:, sl],
                                start=(c == 0 and ah == 0),
                                stop=(c == NCH - 1 and ah == NAH - 1))
                    # ln(nd) -> blk rows [32*rb : 32*rb+4]
                    nc.scalar.activation(blk[32 * rb:32 * rb + 4, :], ndp[:],
                                         AF.Ln, bias=actb[0:4, 0:1])

                    # ---- sim matmuls into the block's chunk PSUMs
                    for c in range(NCH):
                        sl = slice(c * 512, (c + 1) * 512)
                        for ah in range(NAH):
                            nc.tensor.matmul(
                                simcs[c][32 * rb:32 * rb + 32, :],
                                lhsT=qhat[:, ah, b * QLEN:(b + 1) * QLEN],
                                rhs=dts[ah][:, sl],
                                start=(ah == 0), stop=(ah == 1),
                                tile_position=(0, 32 * rb))

                def block_finalize(kb, S, blk, simcs):
                    # pad-bias (+1e4 on padded docs) -> exp underflows to 0
                    nc.vector.tensor_tensor(out=blk[:], in0=blk[:],
                                            in1=pbv[:, kb, :], op=ALU.add)
                    for c in range(NCH):
                        rdbp = prdb.tile([128, 512], f32, tag="rdb",
                                         name="rdbp")
                        nc.tensor.matmul(rdbp[:], lhsT=selv[:, c, :],
                                         rhs=blk[:], start=True, stop=True)
                        rdb = wpool.tile([128, 512], fp16, tag="rdb16",
                                         name="rdb16")
                        nc.scalar.activation(rdb[:], rdbp[:], AF.Exp,
                                             scale=-0.5)
                        nc.vector.tensor_tensor(
                            out=S[:, c * 512:(c + 1) * 512], in0=simcs[c][:],
                            in1=rdb[:], op=ALU.mult)

                def rbf_work(kb, S, scores_t):
                    red = fpool.tile([128, 16], f32, tag="red", name="red")
                    s2 = rpool.tile([128, DLEN], fp16, tag="s2", name="s2")
                    e0 = rpool.tile([128, DLEN], bf16, tag="e0", name="e0")
                    gk2 = rpool.tile([128, DLEN], fp16, tag="gk2", name="gk2")
                    u2d = rpool.tile([128, DLEN], fp16, tag="u2d", name="u2d")
                    e0d = rpool.tile([128, DLEN], fp16, tag="e0d", name="e0d")

                    if direct_k < len(MUS):
                        def head():
                            nc.vector.tensor_tensor(out=s2[:], in0=S[:],
                                                    in1=S[:], op=ALU.mult)
                            nc.scalar.activation(e0[:], s2[:], AF.Exp,
                                                 scale=-50.0)
                        pending.append(head)

                    for k, mu in enumerate(MUS):
                        if k < direct_k:
                            # direct form: Square (Act or DVE) then
                            # Exp+accum on Act; no product pass at all
                            def direct_item(k=k, mu=mu):
                                if k < dsq:
                                    nc.vector.tensor_scalar(
                                        out=e0d[:], in0=S[:], scalar1=-mu,
                                        scalar2=None, op0=ALU.add)
                                    nc.vector.tensor_tensor(
                                        out=u2d[:], in0=e0d[:], in1=e0d[:],
                                        op=ALU.mult)
                                else:
                                    nc.scalar.activation(
                                        u2d[:], S[:], AF.Square,
                                        bias=actb[:, 13 + k:14 + k])
                                nc.scalar.activation(
                                    e0d[:], u2d[:], AF.Exp, scale=-50.0,
                                    accum_out=red[:, k:k + 1])
                            pending.append(direct_item)
                            continue
                        fk = rpool.tile([128, DLEN], bf16, tag="fk", name="fk")
                        gk = rpool.tile([128, DLEN], fp16, tag="gk", name="gk")

                        def mu_item(k=k, mu=mu, fk=fk, gk=gk):
                            # ttr (fused product+reduce) crashes the runtime
                            # through this PJRT path - use 2 passes, with the
                            # reduce on DVE or riding an Act Copy accumulator
                            nc.scalar.activation(fk[:], S[:], AF.Exp,
                                                 scale=100.0 * mu,
                                                 bias=actb[:, 3 + k:4 + k])
                            nc.vector.tensor_tensor(
                                out=gk[:], in0=e0[:], in1=fk[:],
                                op=ALU.mult)
                            if k < red_split:
                                nc.scalar.activation(
                                    gk2[:], gk[:], AF.Copy,
                                    accum_out=red[:, k:k + 1])
                            elif red2:
                                # 2-stage: fp16 pair-add at 2x, then reduce
                                nc.vector.tensor_tensor(
                                    out=gk2[:, 0:1024], in0=gk[:, 0:1024],
                                    in1=gk[:, 1024:2048], op=ALU.add)
                                nc.vector.tensor_reduce(
                                    out=red[:, k:k + 1], in_=gk2[:, 0:1024],
                                    axis=AX.X, op=ALU.add)
                            else:
                                nc.vector.tensor_reduce(
                                    out=red[:, k:k + 1], in_=gk[:],
                                    axis=AX.X, op=ALU.add)
                        pending.append(mu_item)

                    u2 = rpool.tile([128, DLEN], fp16, tag="u2", name="u2")
                    e10 = rpool.tile([128, DLEN], fp16, tag="e10", name="e10")

                    def sharp():
                        nc.scalar.activation(u2[:], S[:], AF.Square,
                                             bias=actb[:, 2:3])
                        if "noacc" in ablate:
                            nc.scalar.activation(e10[:], u2[:], AF.Exp,
                                                 scale=-500000.0)
                            nc.vector.tensor_reduce(
                                out=red[:, 10:11], in_=e10[:], axis=AX.X,
                                op=ALU.add)
                        else:
                            nc.scalar.activation(e10[:], u2[:], AF.Exp,
                                                 scale=-500000.0,
                                                 accum_out=red[:, 10:11])
                        if red_split > 10:
                            nc.scalar.activation(gk2[:], S[:], AF.Copy,
                                                 accum_out=red[:, 11:12])
                        elif red2:
                            nc.vector.tensor_tensor(
                                out=gk2[:, 0:1024], in0=S[:, 0:1024],
                                in1=S[:, 1024:2048], op=ALU.add)
                            nc.vector.tensor_reduce(
                                out=red[:, 11:12], in_=gk2[:, 0:1024],
                                axis=AX.X, op=ALU.add)
                        else:
                            nc.vector.tensor_reduce(out=red[:, 11:12],
                                                    in_=S[:], axis=AX.X,
                                                    op=ALU.add)
                    pending.append(sharp)

                    def tail():
                        eq = fpool.tile([128, 1], f32, tag="eq", name="eq")
                        nc.vector.tensor_scalar(out=eq[:], in0=red[:, 11:12],
                                                scalar1=0.0, scalar2=None,
                                                op0=ALU.is_equal)
                        w = fpool.tile([128, 1], f32, tag="w", name="w")
                        nc.vector.tensor_scalar(out=w[:], in0=eq[:],
                                                scalar1=-1.0, scalar2=1.0,
                                                op0=ALU.mult, op1=ALU.add)
                        lg = fpool.tile([128, 11], fp16, tag="lg", name="lg")
                        nc.scalar.activation(lg[:], red[:, 0:11], AF.Ln,
                                             bias=actb[:, 1:2])
                        p1 = fpool.tile([128, 11], fp16, tag="p1", name="p1")
                        nc.vector.tensor_tensor(out=p1[:], in0=lg[:],
                                                in1=wcb, op=ALU.mult)
                        dot = fpool.tile([128, 1], f32, tag="dot", name="dot")
                        nc.vector.reduce_sum(out=dot[:], in_=p1[:], axis=AX.X)
                        mdot = fpool.tile([128, 1], f32, tag="mdot",
                                          name="mdot")
                        nc.vector.tensor_tensor(out=mdot[:], in0=dot[:],
                                                in1=w[:], op=ALU.mult)
                        sc_ps = pnd.tile([1, 4], f32, tag="ndp", name="sc_ps")
                        nc.tensor.matmul(sc_ps[:], lhsT=mdot[:], rhs=bmask,
                                         start=True, stop=True)
                        nc.vector.tensor_scalar(
                            out=scores_t[:, kb * PACK:(kb + 1) * PACK],
                            in0=sc_ps[:], scalar1=bc, scalar2=None,
                            op0=ALU.add)
                    pending.append(tail)

                # ---------------- main loop: 2 blocks of 4 batches
                for kb in range(BPC // PACK):
                    S = rpool.tile([128, DLEN], fp16, tag="S", name="S")
                    blk = wpool.tile([128, 512], fp16, tag="blk", name="blk")
                    # zero-fill: sel-matmul contracts over all 128 rows and
                    # 0 * garbage(inf/nan) would poison the broadcast
                    nc.gpsimd.memset(blk[:], 0.0)
                    simcs = [psim.tile([128, 512], f32, tag=f"sim{c}",
                                       name=f"sim{c}") for c in range(NCH)]
                    per_batch = -(-len(pending) // PACK) if pending else 0
                    for rb in range(PACK):
                        batch_work(kb * PACK + rb, rb, S, blk, simcs)
                        drain(per_batch)
                    block_finalize(kb, S, blk, simcs)
                    if "rbf" not in ablate:
                        rbf_work(kb, S, scores)

                def outdma(scores_t=scores):
                    nc.sync.dma_start(out=d_out.ap(), in_=scores_t[:])
                pending.append(outdma)

            drain(len(pending))

    try:
        nc.compile()
    finally:
        hw_specs.get_activation_tables = _orig_tables
        _bacc_mod.get_activation_tables = _orig_tables
    return nc


# ---------------------------------------------------------------- host prep
def _proj_table(embed_table, W_t, b_t, b_e):
    proj = embed_table.astype(np.float32) @ W_t.astype(np.float32)
    proj += (np.asarray(b_t, np.float32) + np.asarray(b_e, np.float32))
    return proj


def _prep_core(core, proj, query_tok, doc_tok, query_entity, doc_entity,
               W_e, W_c, b_c):
    bs = slice(core * BPC, (core + 1) * BPC)
    qt = np.asarray(query_tok[bs], dtype=np.int64) + 1   # [8, 32]
    dt = np.asarray(doc_tok[bs], dtype=np.int64) + 1     # [8, 2048]

    uniq, dinv = np.unique(np.concatenate([dt.ravel(), qt.ravel()]),
                           return_inverse=True)
    nuniq = len(uniq)
    assert nuniq <= 17000
    tab = np.zeros((17000, ATT), dtype=np.float16)
    tab[:nuniq] = proj[uniq].astype(np.float16)
    didx = dinv[:BPC * DLEN].reshape(BPC, DLEN).astype(np.int16)
    qidx = dinv[BPC * DLEN:].reshape(BPC * QLEN).astype(np.int16)

    def wrap(a):  # [n] -> [128, n//16] wrapped in 16 partitions, replicated x8
        w = a.reshape(-1, 16).T
        return np.tile(w, (8, 1)).copy()

    idxd = np.concatenate([wrap(didx[b]) for b in range(BPC)], axis=1)
    idxq = wrap(qidx)
    ci16 = np.ascontiguousarray(np.concatenate([idxd, idxq], axis=1))

    entT = np.ascontiguousarray(
        np.asarray(doc_entity[bs], dtype=np.float16).transpose(0, 2, 1))
    qentT = np.ascontiguousarray(
        np.asarray(query_entity[bs], dtype=np.float16).transpose(0, 2, 1)
    ).transpose(1, 0, 2).reshape(128, BPC * QLEN)

    we = W_e.astype(np.float16)                      # [128, 256]
    iden = np.eye(128, dtype=np.float16)
    onescol = np.ones((128, 128), np.float16)
    csel = np.zeros((128, 4, 4), np.float16)
    for c in range(4):
        csel[:, c, c] = 1.0
    csel = csel.reshape(128, 16)
    # sel_c[i, p] = 1 iff i == 32*(p//32) + c   (blk rows at 32*rb + c)
    sel = np.zeros((128, 4, 128), np.float16)
    for c in range(4):
        for p in range(128):
            sel[32 * (p // 32) + c, c, p] = 1.0
    sel = sel.reshape(128, 512)
    # pad-bias per block: pb[32*rb+c, kb, j] = 1e4 if doc padded
    pb = np.zeros((128, 2, 512), np.float16)
    for kb in range(2):
        for rb in range(4):
            for c in range(4):
                padded = (dt[kb * 4 + rb, c * 512:(c + 1) * 512] == 0)
                pb[32 * rb + c, kb, :] = np.where(padded, 1e4, 0.0)
    pb = pb.reshape(128, 1024)
    wcb = np.tile(np.asarray(W_c, dtype=np.float16).reshape(1, 11), (128, 1))
    wcb = np.concatenate(
        [wcb, np.zeros((128, C16_TOT - C16_WC - 11), np.float16)], axis=1)
    cf16 = np.ascontiguousarray(np.concatenate(
        [we, iden, qentT, onescol, csel, sel, pb, wcb], axis=1))
    assert cf16.shape[1] == C16_TOT, cf16.shape

    bmask = np.zeros((128, 4), dtype=np.float32)
    for r in range(4):
        bmask[r * QLEN:(r + 1) * QLEN, r] = 1.0
    actb = np.tile(np.array(
        [1e-30, 1e-6, -1.0] + [-50.0 * m * m for m in MUS]
        + [-m for m in MUS], dtype=np.float32), (128, 1))
    cf32 = np.ascontiguousarray(np.concatenate([bmask, actb], axis=1))
    assert cf32.shape[1] == C32_TOT, cf32.shape

    qmask = (qt != 0).astype(np.float32).reshape(1, BPC * QLEN)
    qrow = np.zeros((1, 257), dtype=np.float32)
    qrow[0, :BPC * QLEN] = qmask
    qrow[0, 256] = np.asarray(b_c, dtype=np.float32).reshape(())

    return {
        "tab": tab, "ci16": ci16, "cf16": cf16, "cf32": cf32, "entT": entT,
        "qrow": qrow, "onesf": np.ones((1, 128), np.float16),
    }


def kernel(**inputs):
    from concourse import bass_utils

    if "nc" not in _CACHE:
        _CACHE["nc"] = _build_program()
    nc = _CACHE["nc"]

    args = {k: np.asarray(v) for k, v in inputs.items()}
    proj = _proj_table(args["embed_table"], args["W_t"], args["b_t"],
                       args["b_e"])
    in_maps = [
        _prep_core(c, proj, args["query_tok"], args["doc_tok"],
                   args["query_entity"], args["doc_entity"],
                   args["W_e"], args["W_c"], args["b_c"])
        for c in range(NCORES)
    ]
    res = bass_utils.run_bass_kernel_spmd(nc, in_maps,
                                          core_ids=list(range(NCORES)))
    out = np.concatenate([res.results[c]["out"].reshape(BPC)
                          for c in range(NCORES)])
    return out.reshape(B, 1).astype(np.float32)
